# revision 4
# baseline (speedup 1.0000x reference)
"""CrossTransformer score kernel for TRN2 (8 NeuronCores, data-parallel over batch).

Reference computation (per batch sample b, with k=n=5, c=512, h=w=7, dk=dv=128):
  s      : (25, 512, 49) support features
  avg    = mean_hw(s)                       # per-support channel means
  sf     = w_s2 @ (w_s1 @ [s; avg_bcast])   # two 1x1 convs -> (25, 64, 49)
  score  = softmax_ch(sigmoid(sf))          # over the 64 channels
  ws     = tile(score, 8x) * s              # gated supports (25, 512, 49)
  q      = w_qk @ query ; qv = w_v @ query  # (128, 49)
  sk     = w_qk @ ws ; sv = w_v @ ws
  sim    = q^T sk * dk^-0.5                 # (k, 49, 245)
  attn   = softmax_nij(sim)
  out    = attn @ sv^T                      # (k, 128, 49)
  result = -sum((qv - out)^2) / 49          # (k,)

Device-side algebra (avoids any partition-dim softmax):
  e      = exp(sigmoid(sf))                 # unnormalized score, in (1, e)
  colsum = sum_ch(e); inv = 1/colsum        # per-column normalizer
  ws_u   = rep2(e) * s                      # unnormalized gated supports
  sim    = (qt^T ws_u) * inv                # qt = SCALE * W_qk^T q (per-col inv)
  attn2  = (exp(sim) * inv) / rowsum(exp(sim))
  out^T  = attn2^T^T @ sv_u^T               # sv_u^T = ws_u^T W_v^T (per-col inv in attn2)
"""

import json
import os
from contextlib import ExitStack

import numpy as np

import concourse.bass as bass
import concourse.tile as tile
from concourse import mybir
from concourse.bass_utils import run_bass_kernel_spmd


def _legalize_sync(raw: bytes) -> bytes:
    """This container's walrus supports very few sync-wait slots per
    instruction (LDWEIGHTS/MATMULT share a single slot; Drain overflows
    too).  Move excess on_wait entries onto standalone EventSemaphore
    instructions injected just before the over-limit instruction on the
    same engine — identical semantics (the engine stalls one instruction
    earlier), raw-bass `wait_ge` encoding."""
    m = json.loads(raw)
    ctr = 0
    for fn in m.get("functions", []):
        for blk in fn.get("blocks", []):
            insts = blk.get("instructions", [])
            out = []
            for inst in insts:
                si = inst.get("sync_info")
                waits = (si or {}).get("on_wait", [])
                op = inst.get("opcode", "")
                allowed = 0 if op in ("Matmult", "MatmultMx", "Ldweights") else 1
                if si is not None and len(waits) > allowed:
                    for w in waits[allowed:]:
                        out.append({
                            "debug": inst.get("debug", 0),
                            "engine": inst["engine"],
                            "ins": [],
                            "outs": [],
                            "name": f"lgw{ctr}",
                            "opcode": "EventSemaphore",
                            "sync_info": {"on_update": [], "on_wait": [w]},
                        })
                        ctr += 1
                    si["on_wait"] = waits[:allowed]
                out.append(inst)
            blk["instructions"] = out
    return json.dumps(m).encode()

F32 = mybir.dt.float32
P = 128

B, K, N, C, H, W = 64, 5, 5, 512, 7, 7
HW = H * W            # 49
KN = K * N            # 25
COLS = KN * HW        # 1225
NIJ = N * HW          # 245
NCH = C // P          # 4 channel chunks
DK = 128              # key/value projection dim
NCORES = 8
BPC = B // NCORES     # 8 samples per core
SCALE = DK ** -0.5

# free-dim chunking of the 1225 columns (PSUM bank = 512 fp32)
NSPLITS = [(0, 512), (512, 512), (1024, 201)]
# nij row-chunking within one k-group (245 rows -> two partition tiles)
RSPLITS = [(0, 128), (128, 117)]

_NC_CACHE = {}


def _build_nc():
    if "nc" in _NC_CACHE:
        return _NC_CACHE["nc"]

    nc = bass.Bass("TRN2", target_bir_lowering=False, debug=False)

    s_in = nc.dram_tensor("s_in", (BPC, NCH, P, COLS), F32, kind="ExternalInput").ap()
    q_in = nc.dram_tensor("q_in", (BPC, NCH, P, HW), F32, kind="ExternalInput").ap()
    w_qkT = nc.dram_tensor("w_qkT", (NCH, P, DK), F32, kind="ExternalInput").ap()
    w_qk_s = nc.dram_tensor("w_qk_s", (DK, C), F32, kind="ExternalInput").ap()
    w_vT = nc.dram_tensor("w_vT", (NCH, P, DK), F32, kind="ExternalInput").ap()
    w_s1aT = nc.dram_tensor("w_s1aT", (NCH, P, 64), F32, kind="ExternalInput").ap()
    w_s1bT = nc.dram_tensor("w_s1bT", (NCH, P, 64), F32, kind="ExternalInput").ap()
    w_s2T = nc.dram_tensor("w_s2T", (64, 64), F32, kind="ExternalInput").ap()
    ident49 = nc.dram_tensor("ident49", (HW, HW), F32, kind="ExternalInput").ap()
    out_d = nc.dram_tensor("out", (BPC, K), F32, kind="ExternalOutput").ap()

    with ExitStack() as ctx:
        tc = ctx.enter_context(tile.TileContext(nc))
        _body(ctx, tc, s_in, q_in, w_qkT, w_qk_s, w_vT, w_s1aT, w_s1bT, w_s2T,
              ident49, out_d)

    patched = _legalize_sync(nc.to_json_bytes())
    nc.to_json_bytes = lambda: patched

    _NC_CACHE["nc"] = nc
    return nc


def _body(ctx, tc, s_in, q_in, w_qkT, w_qk_s, w_vT, w_s1aT, w_s1bT, w_s2T,
          ident49, out_d):
    nc = tc.nc
    Alu = mybir.AluOpType
    Act = mybir.ActivationFunctionType
    Ax = mybir.AxisListType

    consts = ctx.enter_context(tc.tile_pool(name="consts", bufs=1))
    sbig = ctx.enter_context(tc.tile_pool(name="sbig", bufs=2))
    swork = ctx.enter_context(tc.tile_pool(name="swork", bufs=2))
    pbig = ctx.enter_context(tc.tile_pool(name="pbig", bufs=2, space="PSUM"))
    psmall = ctx.enter_context(tc.tile_pool(name="psmall", bufs=2, space="PSUM"))

    # --- constants ---
    w_qkT_t = consts.tile([P, NCH, DK], F32)
    nc.sync.dma_start(out=w_qkT_t, in_=w_qkT.rearrange("c p f -> p c f"))
    w_qk_s_t = consts.tile([DK, C], F32)
    nc.sync.dma_start(out=w_qk_s_t, in_=w_qk_s)
    w_vT_t = consts.tile([P, NCH, DK], F32)
    nc.sync.dma_start(out=w_vT_t, in_=w_vT.rearrange("c p f -> p c f"))
    w_s1aT_t = consts.tile([P, NCH, 64], F32)
    nc.sync.dma_start(out=w_s1aT_t, in_=w_s1aT.rearrange("c p f -> p c f"))
    w_s1bT_t = consts.tile([P, NCH, 64], F32)
    nc.sync.dma_start(out=w_s1bT_t, in_=w_s1bT.rearrange("c p f -> p c f"))
    w_s2T_t = consts.tile([64, 64], F32)
    nc.sync.dma_start(out=w_s2T_t, in_=w_s2T)

    identity_t = consts.tile([HW, HW], F32)
    nc.sync.dma_start(out=identity_t, in_=ident49)
    halves_t = consts.tile([P, HW], F32)   # 0.5: colsum over duplicated e2
    nc.vector.memset(halves_t, 0.5)
    ones49_t = consts.tile([HW, 1], F32)
    nc.vector.memset(ones49_t, 1.0)
    dsum_t = consts.tile([HW, BPC * K], F32)  # per-partition partial dists

    for i in range(BPC):
        # ---- loads ----
        s_t = sbig.tile([P, NCH, COLS], F32, tag="s")
        nc.sync.dma_start(out=s_t, in_=s_in[i].rearrange("c p f -> p c f"))
        q_t = swork.tile([P, NCH, HW], F32, tag="q_in")
        nc.sync.dma_start(out=q_t, in_=q_in[i].rearrange("c p f -> p c f"))

        # ---- score path ----
        # channel means (1/49 folded into w_s1bT on host)
        avg = swork.tile([P, NCH, KN], F32, tag="avg")
        nc.vector.tensor_reduce(
            out=avg,
            in_=s_t.rearrange("p c (s f) -> p c s f", f=HW),
            axis=Ax.X,
            op=Alu.add,
        )
        # part2[64, 25] = w_s1b^T @ avg
        p2 = psmall.tile([64, KN], F32, tag="ps")
        for ci in range(NCH):
            nc.tensor.matmul(
                p2, lhsT=w_s1bT_t[:, ci], rhs=avg[:, ci],
                start=(ci == 0), stop=(ci == NCH - 1),
            )
        p2sb = swork.tile([64, KN], F32, tag="p2")
        nc.any.tensor_copy(out=p2sb, in_=p2)

        # part1[64, 1225] = w_s1a^T @ s
        p1 = pbig.tile([64, COLS], F32, tag="pbig")
        for (n0, nn) in NSPLITS:
            for ci in range(NCH):
                nc.tensor.matmul(
                    p1[:, n0:n0 + nn], lhsT=w_s1aT_t[:, ci],
                    rhs=s_t[:, ci, n0:n0 + nn],
                    start=(ci == 0), stop=(ci == NCH - 1),
                )
        # sf1 = part1 + bcast(part2)
        sf1 = swork.tile([64, COLS], F32, tag="sf1")
        nc.vector.tensor_tensor(
            sf1.rearrange("p (s f) -> p s f", f=HW),
            p1.rearrange("p (s f) -> p s f", f=HW),
            p2sb[:, :, None].to_broadcast((64, KN, HW)),
            Alu.add,
        )
        # sf2 = w_s2^T @ sf1
        sf2 = pbig.tile([64, COLS], F32, tag="pbig")
        for (n0, nn) in NSPLITS:
            nc.tensor.matmul(
                sf2[:, n0:n0 + nn], lhsT=w_s2T_t, rhs=sf1[:, n0:n0 + nn],
                start=True, stop=True,
            )
        # e2 = exp(sigmoid(sf2)), duplicated to both partition halves
        e2 = swork.tile([P, COLS], F32, tag="e2")
        nc.scalar.activation(out=e2[:64], in_=sf2, func=Act.Sigmoid)
        nc.scalar.activation(out=e2[:64], in_=e2[:64], func=Act.Exp)
        nc.sync.dma_start(out=e2[64:128], in_=e2[:64])
        # colsum broadcast to 49 partitions: 0.5 * ones(128,49)^T @ e2
        cs = pbig.tile([HW, COLS], F32, tag="pbig")
        for (n0, nn) in NSPLITS:
            nc.tensor.matmul(
                cs[:, n0:n0 + nn], lhsT=halves_t, rhs=e2[:, n0:n0 + nn],
                start=True, stop=True,
            )
        inv49 = swork.tile([HW, COLS], F32, tag="inv")
        nc.vector.reciprocal(out=inv49, in_=cs)

        # ws_u = e2 * s (unnormalized gated supports)
        ws_t = sbig.tile([P, NCH, COLS], F32, tag="ws")
        for ci in range(NCH):
            nc.vector.tensor_mul(ws_t[:, ci], s_t[:, ci], e2)

        # ---- projections ----
        # q = W_qk @ query
        pq = psmall.tile([P, HW], F32, tag="ps")
        for ci in range(NCH):
            nc.tensor.matmul(
                pq, lhsT=w_qkT_t[:, ci], rhs=q_t[:, ci],
                start=(ci == 0), stop=(ci == NCH - 1),
            )
        q_sb = swork.tile([P, HW], F32, tag="qsb")
        nc.any.tensor_copy(out=q_sb, in_=pq)
        # qt = SCALE * W_qk^T q  (SCALE folded into w_qk_s on host)
        pqt = psmall.tile([P, NCH, HW], F32, tag="ps")
        for ci in range(NCH):
            nc.tensor.matmul(
                pqt[:, ci], lhsT=w_qk_s_t[:, ci * P:(ci + 1) * P], rhs=q_sb,
                start=True, stop=True,
            )
        qt_sb = swork.tile([P, NCH, HW], F32, tag="qt")
        nc.any.tensor_copy(out=qt_sb, in_=pqt)
        # qv^T[49, 128] = query^T @ W_v^T
        pqv = psmall.tile([HW, DK], F32, tag="ps")
        for ci in range(NCH):
            nc.tensor.matmul(
                pqv, lhsT=q_t[:, ci], rhs=w_vT_t[:, ci],
                start=(ci == 0), stop=(ci == NCH - 1),
            )
        qvT_sb = swork.tile([HW, DK], F32, tag="qvT")
        nc.any.tensor_copy(out=qvT_sb, in_=pqv)

        # sim_u[49, 1225] = qt^T @ ws_u
        sim_p = pbig.tile([HW, COLS], F32, tag="pbig")
        for (n0, nn) in NSPLITS:
            for ci in range(NCH):
                nc.tensor.matmul(
                    sim_p[:, n0:n0 + nn], lhsT=qt_sb[:, ci],
                    rhs=ws_t[:, ci, n0:n0 + nn],
                    start=(ci == 0), stop=(ci == NCH - 1),
                )
        # sv_u^T[nij, 128] = ws_u^T @ W_v^T, stored per (k, half)
        svT_sb = swork.tile([P, K, 2, DK], F32, tag="svT")
        for k in range(K):
            for h, (r0, rn) in enumerate(RSPLITS):
                psv = psmall.tile([P, DK], F32, tag="ps")
                for ci in range(NCH):
                    nc.tensor.matmul(
                        psv[:rn], lhsT=ws_t[:, ci, NIJ * k + r0:NIJ * k + r0 + rn],
                        rhs=w_vT_t[:, ci],
                        start=(ci == 0), stop=(ci == NCH - 1),
                    )
                nc.any.tensor_copy(out=svT_sb[:rn, k, h], in_=psv[:rn])

        # ---- attention ----
        sim_sb = swork.tile([HW, COLS], F32, tag="sim")
        nc.vector.tensor_mul(sim_sb, sim_p, inv49)
        nc.scalar.activation(out=sim_sb, in_=sim_sb, func=Act.Exp)  # e, in place
        rs = swork.tile([HW, K], F32, tag="rs")
        nc.vector.tensor_reduce(
            out=rs, in_=sim_sb.rearrange("p (k f) -> p k f", f=NIJ),
            axis=Ax.X, op=Alu.add,
        )
        rsi = swork.tile([HW, K], F32, tag="rsi")
        nc.vector.reciprocal(out=rsi, in_=rs)
        f_sb = swork.tile([HW, COLS], F32, tag="fsb")
        nc.vector.tensor_mul(f_sb, sim_sb, inv49)  # e * inv
        nc.vector.tensor_tensor(                    # attn2 = e * inv / rowsum
            f_sb.rearrange("p (k f) -> p k f", f=NIJ),
            f_sb.rearrange("p (k f) -> p k f", f=NIJ),
            rsi[:, :, None].to_broadcast((HW, K, NIJ)),
            Alu.mult,
        )

        # per k: transpose attn2, multiply with sv^T, distance
        at_sb = swork.tile([P, K, 2, HW], F32, tag="at")
        for k in range(K):
            for h, (r0, rn) in enumerate(RSPLITS):
                pat = psmall.tile([P, HW], F32, tag="ps")
                nc.tensor.transpose(
                    pat[:rn], f_sb[:, NIJ * k + r0:NIJ * k + r0 + rn], identity_t
                )
                nc.any.tensor_copy(out=at_sb[:rn, k, h], in_=pat[:rn])
            po = psmall.tile([HW, DK], F32, tag="ps")
            for h, (r0, rn) in enumerate(RSPLITS):
                nc.tensor.matmul(
                    po, lhsT=at_sb[:rn, k, h], rhs=svT_sb[:rn, k, h],
                    start=(h == 0), stop=(h == 1),
                )
            dk_sb = swork.tile([HW, DK], F32, tag="dk")
            nc.vector.tensor_sub(dk_sb, qvT_sb, po)
            nc.scalar.activation(
                out=dk_sb, in_=dk_sb, func=Act.Square,
                accum_out=dsum_t[:, i * K + k:i * K + k + 1],
            )

    # ---- final partition reduction: (49, 40) -> (1, 40), scale by -1/49 ----
    pfin = psmall.tile([1, BPC * K], F32, tag="ps")
    nc.tensor.matmul(pfin, lhsT=ones49_t, rhs=dsum_t, start=True, stop=True)
    fin_sb = swork.tile([1, BPC * K], F32, tag="fin")
    nc.scalar.mul(fin_sb, pfin, -1.0 / HW)
    nc.sync.dma_start(out=out_d.rearrange("b k -> (b k)")[None, :], in_=fin_sb)


def _prep_inputs(query_repr, supports_repr, w_qk, w_v, w_s1, w_s2):
    q = np.ascontiguousarray(np.asarray(query_repr, dtype=np.float32))
    s = np.ascontiguousarray(np.asarray(supports_repr, dtype=np.float32))
    w_qk = np.asarray(w_qk, dtype=np.float32)
    w_v = np.asarray(w_v, dtype=np.float32)
    w_s1 = np.asarray(w_s1, dtype=np.float32)
    w_s2 = np.asarray(w_s2, dtype=np.float32)

    # (b, kn, c, hw) -> (b, nch, 128, kn*hw)
    s_host = np.ascontiguousarray(
        s.reshape(B, KN, NCH, P, HW).transpose(0, 2, 3, 1, 4)
    ).reshape(B, NCH, P, COLS)
    q_host = np.ascontiguousarray(q.reshape(B, NCH, P, HW))

    weights = {
        "w_qkT": np.ascontiguousarray(w_qk.T).reshape(NCH, P, DK),
        "w_qk_s": np.ascontiguousarray(w_qk * np.float32(SCALE)),
        "w_vT": np.ascontiguousarray(w_v.T).reshape(NCH, P, DK),
        "w_s1aT": np.ascontiguousarray(w_s1[:, :C].T).reshape(NCH, P, 64),
        "w_s1bT": np.ascontiguousarray((w_s1[:, C:] / np.float32(HW)).T).reshape(NCH, P, 64),
        "w_s2T": np.ascontiguousarray(w_s2.T),
        "ident49": np.eye(HW, dtype=np.float32),
    }
    return s_host, q_host, weights


LAST_RESULT = None


def kernel(query_repr, supports_repr, w_qk, w_v, w_s1, w_s2, _trace=False):
    global LAST_RESULT
    s_host, q_host, weights = _prep_inputs(
        query_repr, supports_repr, w_qk, w_v, w_s1, w_s2
    )
    nc = _build_nc()

    in_maps = []
    for c in range(NCORES):
        m = dict(weights)
        m["s_in"] = np.ascontiguousarray(s_host[c * BPC:(c + 1) * BPC])
        m["q_in"] = np.ascontiguousarray(q_host[c * BPC:(c + 1) * BPC])
        in_maps.append(m)

    res = run_bass_kernel_spmd(
        nc, in_maps, core_ids=list(range(NCORES)), trace=_trace
    )
    LAST_RESULT = res
    out = np.concatenate([r["out"] for r in res.results], axis=0)
    return out.astype(np.float32)
